# revision 1
# baseline (speedup 1.0000x reference)
"""2-layer GATv2 (N=50000, E=1.6M, D=H=128, O=64) on 8 trn2 NeuronCores.

v2 strategy (1D dst partition, 49 blocks of 128 dst per core):
- Source features fetched with bulk `dma_gather` (1024 rows/instruction,
  int16 indices; src id space split at 32768 into lo/hi tables).
- Target features ALSO gathered (per-edge) from a per-core xr table, so the
  one-hot expansion matmuls and their PSUM round-trips disappear.
- att is folded into the tables (Wl*att, Wr*att, feature-permuted so
  att>0 features come first); leaky+dot becomes two scalar_tensor_tensor
  half-passes (max / min forms) + one grouped reduce. The fold is undone
  after aggregation by a fused (U*r)*inv_att stt from PSUM.
- Segment softmax: seg_p[e,d] = exp(mask[e,d] + e_logit[e]) computed on the
  scalar engine from a host-built fp8 {0,-60} one-hot mask, per 128-edge
  tile, fused with the exp. Aggregation is tensor-engine matmuls
  (lhsT=seg_p, rhs=[gathered g | ones]) accumulating U and the softmax
  denominator in PSUM.
- Layer-2 source table is AllGather'ed between layers.
"""
import json
import sys

import ml_dtypes
import numpy as np

import concourse.bass as bass
import concourse.mybir as mybir
import concourse.tile as tile
from concourse.masks import make_identity

F32 = mybir.dt.float32
BF16 = mybir.dt.bfloat16
FP8 = mybir.dt.float8e4
I16 = mybir.dt.int16
AL = mybir.AluOpType
ACTF = mybir.ActivationFunctionType
NEG = 0.2
F8 = ml_dtypes.float8_e4m3fn
BF = ml_dtypes.bfloat16

# ---------------------------------------------------------------------------
# environment fixups (walrus single-sync-wait limit)
# ---------------------------------------------------------------------------
_SPLIT_SEQ = [0]


def _split_multi_waits_json(m):
    for fn in m.get("functions", []):
        for bb in fn.get("blocks", []):
            insts = bb.get("instructions")
            if not insts:
                continue
            out = []
            for inst in insts:
                si = inst.get("sync_info")
                waits = si.get("on_wait") if si else None
                if waits and len(waits) > 1:
                    for w in waits[:-1]:
                        _SPLIT_SEQ[0] += 1
                        out.append({
                            "debug": inst.get("debug", 0),
                            "engine": inst["engine"],
                            "ins": [], "outs": [],
                            "name": f"waitsplit-{_SPLIT_SEQ[0]}",
                            "opcode": "NoOp",
                            "sync_info": {"on_update": [], "on_wait": [w]},
                        })
                    si["on_wait"] = [waits[-1]]
                out.append(inst)
            bb["instructions"] = out
    return m


_FIXED = [False]


def _install_fixups():
    if _FIXED[0]:
        return
    _FIXED[0] = True
    orig = bass.Bass.to_json_bytes

    def patched(self, *a, **k):
        return json.dumps(
            _split_multi_waits_json(json.loads(orig(self, *a, **k)))
        ).encode()

    bass.Bass.to_json_bytes = patched


# ---------------------------------------------------------------------------
# problem constants
# ---------------------------------------------------------------------------
N, E, D, H, O = 50000, 1_600_000, 128, 128, 64
NC, NBLK = 8, 49
NOWN = NBLK * 128            # 6272
NPAD = NC * NOWN             # 50176
LOS = 32768                  # src id split (= 256 preamble blocks)
NHI = NPAD - LOS             # 17408
GN = 1024                    # dma_gather chunk (rows per instruction)


def _wrap_chunks(flat):
    """int16 index list -> [128, n/16] dma_gather layout, chunked by GN.
    Within each chunk, index j sits at [j%16, j//16], replicated 8x down
    the partitions."""
    outs = []
    for k0 in range(0, len(flat), GN):
        ch = flat[k0:k0 + GN]
        w16 = np.ascontiguousarray(ch.reshape(-1, 16).T)
        outs.append(np.tile(w16, (8, 1)))
    return np.ascontiguousarray(np.concatenate(outs, axis=1)).astype(np.int16)


def _host_prep(x, edge_index, w, TB=None):
    src, dst = np.asarray(edge_index[0]), np.asarray(edge_index[1])
    x = np.asarray(x, np.float32)
    xpad = np.zeros((NPAD, D), np.float32)
    xpad[:N] = x

    att1 = np.asarray(w["att1"], np.float32)
    att2 = np.asarray(w["att2"], np.float32)
    p1 = np.argsort(att1 <= 0, kind="stable")
    p2 = np.argsort(att2 <= 0, kind="stable")
    fp1 = int((att1 > 0).sum())
    fp2 = int((att2 > 0).sum())
    a1 = att1[p1]
    a2 = att2[p2]

    Wl1a = (np.asarray(w["Wl1"], np.float32) * att1[None, :])[:, p1]
    Wr1a = (np.asarray(w["Wr1"], np.float32) * att1[None, :])[:, p1]
    Wlin1p = np.asarray(w["Wlin1"], np.float32)[:, p1]
    c1a = ((np.asarray(w["bl1"]) + np.asarray(w["br1"])) * att1)[p1]
    s1p = (np.asarray(w["blin1"]) + np.asarray(w["bias1"])
           + np.asarray(w["bl1"]))[p1]
    # layer-2 weights take permuted-h rows
    Wl2a = ((np.asarray(w["Wl2"], np.float32) * att2[None, :])[:, p2])[p1, :]
    Wr2a = ((np.asarray(w["Wr2"], np.float32) * att2[None, :])[:, p2])[p1, :]
    Wlin2p = (np.asarray(w["Wlin2"], np.float32)[:, p2])[p1, :]
    c2a = ((np.asarray(w["bl2"]) + np.asarray(w["br2"])) * att2)[p2]
    s2p = (np.asarray(w["blin2"]) + np.asarray(w["bias2"])
           + np.asarray(w["bl2"]))[p2]

    # per-core / per-block edge layout (lo-src first, then hi-src)
    per_core = []
    max_lo, max_hi = 0, 0
    for c in range(NC):
        m = (dst // NOWN) == c
        s_c, d_c = src[m], dst[m] - c * NOWN
        blocks = []
        for b in range(NBLK):
            mb = (d_c // 128) == b
            s_b, drel_b = s_c[mb], d_c[mb] - b * 128
            lo = s_b < LOS
            sl, dl = s_b[lo], drel_b[lo]
            sh, dh = s_b[~lo], drel_b[~lo]
            max_lo = max(max_lo, len(sl))
            max_hi = max(max_hi, len(sh))
            blocks.append((sl, dl, sh, dh))
        per_core.append(blocks)
    T_lo = (max_lo + 127) // 128
    T_hi = (max_hi + 127) // 128
    if TB is None:
        TB = (T_lo, T_hi, fp1, fp2)
    assert T_lo <= TB[0] and T_hi <= TB[1] and fp1 == TB[2] and fp2 == TB[3]
    T_lo, T_hi = TB[0], TB[1]
    T = T_lo + T_hi

    shared = {
        "xT": np.ascontiguousarray(xpad.T).astype(BF),
        "W1cat": np.concatenate([Wl1a, Wr1a, Wlin1p], axis=1).astype(BF),
        "W2cat": np.concatenate([Wl2a, Wr2a, Wlin2p], axis=1).astype(BF),
        "c1a_rep": np.tile(c1a.astype(np.float32), (128, 1)),
        "s1p_rep": np.tile(s1p.astype(np.float32), (128, 1)),
        "inv1_rep": np.tile((1.0 / a1).astype(np.float32), (128, 1)),
        "c2a_rep": np.tile(c2a.astype(np.float32), (128, 1)),
        "s2p_rep": np.tile(s2p.astype(np.float32), (128, 1)),
        "inv2_rep": np.tile((1.0 / a2).astype(np.float32), (128, 1)),
        "iota_col": np.arange(128, dtype=np.float32)[:, None],
    }

    in_maps = []
    for c in range(NC):
        sidx_lo = np.zeros((NBLK, 128, T_lo * 8), np.int16)
        sidx_hi = np.zeros((NBLK, 128, max(T_hi, 1) * 8), np.int16)
        drel_row = np.full((NBLK, 1, T * 128), -1.0, np.float32)
        maskf8 = np.empty((NBLK, 128, T * 128), F8)
        for b, (sl, dl, sh, dh) in enumerate(per_core[c]):
            fl = np.zeros(T_lo * 128, np.int16)
            fl[:len(sl)] = sl.astype(np.int16)
            sidx_lo[b] = _wrap_chunks(fl)
            fh = np.zeros(T_hi * 128, np.int16)
            fh[:len(sh)] = (sh - LOS).astype(np.int16)
            sidx_hi[b] = _wrap_chunks(fh) if T_hi else 0
            drel = np.full(T * 128, -1, np.int32)
            jl = np.arange(len(sl))
            drel[jl] = dl
            jh = T_lo * 128 + np.arange(len(sh))
            drel[jh] = dh
            drel_row[b, 0] = drel.astype(np.float32)
            mk = np.full((128, T * 128), -60.0, F8)
            j = np.concatenate([jl, jh])
            dd = drel[j]
            mk[j % 128, (j // 128) * 128 + dd] = 0.0
            maskf8[b] = mk
        im = dict(shared)
        im["x_ownT"] = np.ascontiguousarray(
            xpad[c * NOWN:(c + 1) * NOWN].T).astype(BF)
        im["sidx_lo"] = sidx_lo
        im["sidx_hi"] = sidx_hi
        im["drel_row"] = drel_row.astype(BF)
        im["maskf8"] = maskf8
        in_maps.append(im)
    return in_maps, TB, p2


def _build_program(TB):
    T_lo, T_hi, fp1, fp2 = TB
    T = T_lo + T_hi
    NBT = NPAD // 128
    nc = bass.Bass()

    def din(name, shape, dt=F32):
        return nc.dram_tensor(name, shape, dt, kind="ExternalInput")

    xT = din("xT", [D, NPAD], BF16)
    x_ownT = din("x_ownT", [D, NOWN], BF16)
    W1cat = din("W1cat", [D, 3 * H], BF16)
    W2cat = din("W2cat", [H, 3 * O], BF16)
    c1a_rep = din("c1a_rep", [128, H])
    s1p_rep = din("s1p_rep", [128, H])
    inv1_rep = din("inv1_rep", [128, H])
    c2a_rep = din("c2a_rep", [128, O])
    s2p_rep = din("s2p_rep", [128, O])
    inv2_rep = din("inv2_rep", [128, O])
    sidx_lo = din("sidx_lo", [NBLK, 128, T_lo * 8], I16)
    sidx_hi = din("sidx_hi", [NBLK, 128, max(T_hi, 1) * 8], I16)
    drel_row = din("drel_row", [NBLK, 1, T * 128], BF16)
    maskf8 = din("maskf8", [NBLK, 128, T * 128], FP8)
    iota_col = din("iota_col", [128, 1])
    out_own = nc.dram_tensor("out_own", [NOWN, O], F32, kind="ExternalOutput")

    from concourse.library_config import mlp

    with tile.TileContext(nc) as tc:
        with (
            tc.tile_pool(name="dram", bufs=1, space="DRAM") as dram,
            tc.tile_pool(name="const", bufs=1) as cpool,
            tc.tile_pool(name="res", bufs=1) as rpool,
            tc.tile_pool(name="bp", bufs=2) as bpool,
            tc.tile_pool(name="gp", bufs=2) as gpool,
            tc.tile_pool(name="xp", bufs=2) as xpool,
            tc.tile_pool(name="sp", bufs=2) as spool,
            tc.tile_pool(name="mp", bufs=2) as mpool,
            tc.tile_pool(name="sg", bufs=2) as sgpool,
            tc.tile_pool(name="ip", bufs=2) as ipool,
            tc.tile_pool(name="zz", bufs=2) as zpool,
            tc.tile_pool(name="ee", bufs=2) as epool,
            tc.tile_pool(name="tl", bufs=3) as tpool,
            tc.tile_pool(name="pa", bufs=2, space="PSUM") as pa,
            tc.tile_pool(name="pu", bufs=2, space="PSUM") as pu,
            tc.tile_pool(name="pt", bufs=1, space="PSUM") as pt,
            tc.tile_pool(name="pr", bufs=1, space="PSUM") as pr,
            tc.tile_pool(name="px", bufs=2, space="PSUM") as px,
        ):
            nc.gpsimd.load_library(mlp)

            xl1_lo = dram.tile([LOS, 128], BF16)
            xl1_hi = dram.tile([NHI, 128], BF16)
            xl2_own = dram.tile([NOWN, 128], BF16)
            xl2_full = dram.tile([NPAD, 128], BF16)
            xl2_lo = dram.tile([LOS, 128], BF16)
            xl2_hi = dram.tile([NHI, 128], BF16)

            def ld(shape, apsrc, name, dt=F32):
                t = cpool.tile(shape, dt, tag=name, name=name)
                nc.sync.dma_start(out=t[:], in_=apsrc)
                return t

            W1_s = ld([D, 3 * H], W1cat[:], "W1", BF16)
            W2_s = ld([H, 3 * O], W2cat[:], "W2", BF16)
            c1_s = ld([128, H], c1a_rep[:], "c1")
            s1_s = ld([128, H], s1p_rep[:], "s1")
            i1_s = ld([128, H], inv1_rep[:], "i1")
            c2_s = ld([128, O], c2a_rep[:], "c2")
            s2_s = ld([128, O], s2p_rep[:], "s2")
            i2_s = ld([128, O], inv2_rep[:], "i2")
            ones1 = cpool.tile([128, 1], BF16, tag="ones1")
            nc.vector.memset(ones1[:], 1.0)
            onesr = cpool.tile([1, 128], BF16, tag="onesr")
            nc.vector.memset(onesr[:], 1.0)
            ic_s = ld([128, 1], iota_col[:], "ic")
            ident = cpool.tile([128, 128], F32, tag="ident")
            make_identity(nc, ident[:])

            regs = {}
            for nrow in {GN, T_lo * 128 % GN, T_hi * 128 % GN, T * 128 % GN}:
                if nrow:
                    regs[nrow] = nc.gpsimd.to_reg(nrow)

            skip_all = rpool.tile([128, NBLK * H], BF16, tag="skip_all")
            hT_all = rpool.tile([128, NBLK * 128], BF16, tag="hT_all")
            skip2_all = rpool.tile([128, NBLK * O], BF16, tag="skip2_all")
            xr_all = rpool.tile([128, NBLK * H], BF16, tag="xr_all")
            xr2_all = rpool.tile([128, NBLK * O], BF16, tag="xr2_all")

            # ---- layer-1 preamble: xl1 table for all nodes ----
            for blk in range(NBT):
                xtb = bpool.tile([D, 128], BF16, tag="xtb")
                nc.sync.dma_start(out=xtb[:], in_=xT[:, blk * 128:(blk + 1) * 128])
                ps = pa.tile([128, 3 * H], F32, tag="pa")
                nc.tensor.matmul(out=ps[:, 0:H], lhsT=xtb[:], rhs=W1_s[:, 0:H],
                                 start=True, stop=True)
                xlb = bpool.tile([128, 128], BF16, tag="xlb")
                nc.scalar.activation(out=xlb[:], in_=ps[:, 0:H], func=ACTF.Copy)
                if blk < LOS // 128:
                    nc.sync.dma_start(
                        out=xl1_lo[blk * 128:(blk + 1) * 128, :], in_=xlb[:])
                else:
                    r0 = blk * 128 - LOS
                    nc.sync.dma_start(out=xl1_hi[r0:r0 + 128, :], in_=xlb[:])

            # ---- layer-1 own preamble: xr1 table + skip ----
            for b in range(NBLK):
                xob = bpool.tile([D, 128], BF16, tag="xob")
                nc.sync.dma_start(out=xob[:], in_=x_ownT[:, b * 128:(b + 1) * 128])
                ps = pa.tile([128, 3 * H], F32, tag="pa")
                nc.tensor.matmul(out=ps[:], lhsT=xob[:], rhs=W1_s[:],
                                 start=True, stop=True)
                nc.vector.tensor_add(out=xr_all[:, b * H:(b + 1) * H],
                                     in0=ps[:, H:2 * H], in1=c1_s[:])
                skt = bpool.tile([128, H], BF16, tag="skt")
                nc.vector.tensor_add(out=skt[:], in0=ps[:, 2 * H:3 * H],
                                     in1=s1_s[:])
                xru = bpool.tile([128, H], BF16, tag="xru")
                nc.vector.tensor_mul(out=xru[:],
                                     in0=xr_all[:, b * H:(b + 1) * H],
                                     in1=i1_s[:])
                nc.vector.tensor_tensor(out=skip_all[:, b * H:(b + 1) * H],
                                        in0=skt[:], in1=xru[:],
                                        op=AL.subtract)

            def edge_pass(layer):
                F = H if layer == 1 else O
                Fp = fp1 if layer == 1 else fp2
                tbl_lo = xl1_lo if layer == 1 else xl2_lo
                tbl_hi = xl1_hi if layer == 1 else xl2_hi
                xr_src = xr_all if layer == 1 else xr2_all
                inv_s = i1_s if layer == 1 else i2_s
                for b in range(NBLK):
                    il = ipool.tile([128, T_lo * 8], I16, tag="il")
                    nc.sync.dma_start(out=il[:], in_=sidx_lo[b, :, :])
                    ih = ipool.tile([128, max(T_hi, 1) * 8], I16, tag="ih")
                    nc.sync.dma_start(out=ih[:], in_=sidx_hi[b, :, :])
                    drow = ipool.tile([1, T * 128], BF16, tag="drow")
                    nc.sync.dma_start(out=drow[:], in_=drel_row[b, :, :])
                    msk = mpool.tile([128, T * 128], FP8, tag="msk")
                    nc.sync.dma_start(out=msk[:], in_=maskf8[b, :, :])

                    gall = gpool.tile([128, T * 128], BF16, tag="gall")

                    def gathers(tbl, idxt, nrows, base_t):
                        for k0 in range(0, nrows, GN):
                            cn = min(GN, nrows - k0)
                            o0 = base_t * 128 + k0
                            nc.gpsimd.dma_gather(
                                gv_cur[:, o0:o0 + cn].rearrange(
                                    "p (t f) -> p t f", f=128),
                                tbl[:], idxt[:, (k0 // GN) * 64:
                                             (k0 // GN) * 64 + cn // 16],
                                cn, regs[cn if cn < GN else GN], 128)

                    gv_cur = gall
                    gathers(tbl_lo, il, T_lo * 128, 0)
                    if T_hi:
                        gathers(tbl_hi, ih, T_hi * 128, T_lo)

                    # one-hot (dst-partition orientation) via bcast matmul
                    oh = xpool.tile([128, T * 128], BF16, tag="oh")
                    for c0 in range(0, T * 128, 512):
                        cw = min(512, T * 128 - c0)
                        psr = pr.tile([128, 512], F32, tag="pr")
                        nc.tensor.matmul(
                            out=psr[:, :cw], lhsT=onesr[:],
                            rhs=drow[:, c0:c0 + cw], start=True, stop=True)
                        nc.vector.tensor_scalar(
                            out=oh[:, c0:c0 + cw], in0=psr[:, :cw],
                            scalar1=ic_s[:, 0:1], scalar2=None,
                            op0=AL.is_equal)

                    FW = F + 4
                    su = spool.tile([128, T * FW], BF16, tag="su")
                    g3 = gall[:].rearrange("p (t f) -> p t f", f=128)
                    s3 = su[:].rearrange("p (t f) -> p t f", f=FW)
                    nc.vector.memset(s3[:, :, F:F + 1], 1.0)
                    eall = epool.tile([128, T], F32, tag="eall")
                    for g0 in range(0, T, 4):
                        q = min(4, T - g0)
                        psx = px.tile([128, 4 * F], F32, tag="px")
                        for i in range(q):
                            nc.tensor.matmul(
                                out=psx[:, i * F:(i + 1) * F],
                                lhsT=oh[:, (g0 + i) * 128:(g0 + i + 1) * 128],
                                rhs=xr_src[:, b * F:(b + 1) * F],
                                start=True, stop=True)
                        p3 = psx[:].rearrange("p (t f) -> p t f", f=F)
                        nc.vector.tensor_add(
                            out=s3[:, g0:g0 + q, 0:F],
                            in0=g3[:, g0:g0 + q, 0:F], in1=p3[:, 0:q, :])
                        zb = zpool.tile([128, 4 * F], BF16, tag="zb")
                        z3 = zb[:].rearrange("p (t f) -> p t f", f=F)
                        nc.vector.scalar_tensor_tensor(
                            out=z3[:, 0:q, 0:Fp], in0=s3[:, g0:g0 + q, 0:Fp],
                            scalar=NEG, in1=s3[:, g0:g0 + q, 0:Fp],
                            op0=AL.mult, op1=AL.max)
                        nc.vector.scalar_tensor_tensor(
                            out=z3[:, 0:q, Fp:F], in0=s3[:, g0:g0 + q, Fp:F],
                            scalar=NEG, in1=s3[:, g0:g0 + q, Fp:F],
                            op0=AL.mult, op1=AL.min)
                        nc.vector.tensor_reduce(
                            out=eall[:, g0:g0 + q], in_=z3[:, 0:q, :],
                            axis=mybir.AxisListType.X, op=AL.add)

                    segp = sgpool.tile([128, T * 128], BF16, tag="segp")
                    for t in range(T):
                        nc.scalar.activation(
                            out=segp[:, t * 128:(t + 1) * 128],
                            in_=msk[:, t * 128:(t + 1) * 128],
                            func=ACTF.Exp, bias=eall[:, t:t + 1], scale=1.0)

                    U = pu.tile([128, F + 4], F32, tag="pu")
                    for t in range(T):
                        nc.tensor.matmul(
                            out=U[:, 0:F + 1],
                            lhsT=segp[:, t * 128:(t + 1) * 128],
                            rhs=su[:, t * FW:t * FW + F + 1],
                            start=(t == 0), stop=(t == T - 1))
                    rr = tpool.tile([128, 1], F32, tag="rr")
                    nc.vector.reciprocal(out=rr[:], in_=U[:, F:F + 1])
                    t1 = tpool.tile([128, F], BF16, tag="t1")
                    nc.vector.scalar_tensor_tensor(
                        out=t1[:], in0=U[:, 0:F], scalar=rr[:, 0:1],
                        in1=inv_s[:, 0:F], op0=AL.mult, op1=AL.mult)
                    if layer == 1:
                        hp = tpool.tile([128, F], BF16, tag="hp")
                        nc.vector.tensor_add(
                            out=hp[:], in0=t1[:],
                            in1=skip_all[:, b * F:(b + 1) * F])
                        hb = tpool.tile([128, F], F32, tag="hb")
                        nc.scalar.activation(out=hb[:], in_=hp[:], func=ACTF.Relu)
                        pst = pt.tile([128, 128], F32, tag="pt")
                        nc.tensor.transpose(out=pst[:], in_=hb[:], identity=ident[:])
                        nc.scalar.activation(
                            out=hT_all[:, b * 128:(b + 1) * 128], in_=pst[:],
                            func=ACTF.Copy)
                        ps2 = pa.tile([128, 3 * H], F32, tag="pa")
                        nc.tensor.matmul(
                            out=ps2[:, 0:3 * O],
                            lhsT=hT_all[:, b * 128:(b + 1) * 128],
                            rhs=W2_s[:], start=True, stop=True)
                        xl2b = bpool.tile([128, O], BF16, tag="xl2b")
                        nc.scalar.activation(out=xl2b[:], in_=ps2[:, 0:O],
                                             func=ACTF.Copy)
                        nc.sync.dma_start(
                            out=xl2_own[b * 128:(b + 1) * 128, 0:O],
                            in_=xl2b[:])
                        nc.vector.tensor_add(
                            out=xr2_all[:, b * O:(b + 1) * O],
                            in0=ps2[:, O:2 * O], in1=c2_s[:])
                        skt2 = bpool.tile([128, O], BF16, tag="skt2")
                        nc.vector.tensor_add(out=skt2[:],
                                             in0=ps2[:, 2 * O:3 * O],
                                             in1=s2_s[:])
                        xru2 = bpool.tile([128, O], BF16, tag="xru2")
                        nc.vector.tensor_mul(
                            out=xru2[:], in0=xr2_all[:, b * O:(b + 1) * O],
                            in1=i2_s[:])
                        nc.vector.tensor_tensor(
                            out=skip2_all[:, b * O:(b + 1) * O],
                            in0=skt2[:], in1=xru2[:], op=AL.subtract)
                    else:
                        ob = tpool.tile([128, F], F32, tag="ob")
                        nc.vector.tensor_add(
                            out=ob[:], in0=t1[:],
                            in1=skip2_all[:, b * F:(b + 1) * F])
                        nc.sync.dma_start(
                            out=out_own[b * 128:(b + 1) * 128, :], in_=ob[:])

            edge_pass(1)

            nc.gpsimd.collective_compute(
                "AllGather", AL.bypass,
                replica_groups=[list(range(NC))],
                ins=[xl2_own[:].opt()],
                outs=[xl2_full[:].opt()],
            )
            nc.sync.dma_start(out=xl2_lo[:], in_=xl2_full[0:LOS, :])
            nc.sync.dma_start(out=xl2_hi[:], in_=xl2_full[LOS:NPAD, :])

            edge_pass(2)

    from concourse.library_overlay import lower_extended_insts
    lower_extended_insts(nc)
    return nc


_W_KEYS = [
    "Wl1", "bl1", "Wr1", "br1", "att1", "bias1", "Wlin1", "blin1",
    "Wl2", "bl2", "Wr2", "br2", "att2", "bias2", "Wlin2", "blin2",
]


def kernel(x, edge_index, **w):
    _install_fixups()
    from concourse.bass_utils import run_bass_kernel_spmd

    w = {k: np.asarray(w[k], np.float32) for k in _W_KEYS}
    in_maps, TB, p2 = _host_prep(np.asarray(x), np.asarray(edge_index), w)
    nc = _build_program(TB)
    last_err = None
    for attempt in range(3):
        try:
            res = run_bass_kernel_spmd(nc, in_maps, core_ids=list(range(NC)))
            break
        except Exception as exc:  # flaky device recovery
            last_err = exc
            print(f"kernel: attempt {attempt} failed: {exc}", file=sys.stderr)
    else:
        raise last_err
    outp = np.concatenate(
        [res.results[c]["out_own"] for c in range(NC)], axis=0
    )[:N]
    out = np.empty_like(outp)
    out[:, p2] = outp
    return out.astype(np.float32)

